# revision 33
# baseline (speedup 1.0000x reference)
# Grouped GEMM (MoE) kernel for Trainium2, 8 NeuronCores.
#
# Sharding: tensor-parallel over out_features (column parallel). Each core
# computes ALL 4096 tokens against its own 416-column slice of every
# expert's weight. No collectives; host concatenates per-core outputs
# along the feature axis. Perfectly load balanced regardless of the
# (uneven) per-expert token counts; the program is identical on every
# core (SPMD) -- only the weight *values* differ.
#
# Layout: the PE contracts over the partition dim of both operands, so
# both need in_features on partitions. The host pre-transposes x once
# (x_T [in, tokens]); w is already [in, out]. Per 128-token tile of one
# expert's segment:
#   psum[tok, col] += xT_tile[k, tok].T @ w_tile[k, col]   (k accumulated)
#
# Perf structure (vs the naive version):
#  - bf16 operands: PE runs 1 cycle/row (fp32 is 4) and DMA bytes halve.
#  - First expert's W and first x chunk are DMA'd in 4 k-part slices and
#    the first chunk's matmuls run part-outer, so the PE starts ~3.5us
#    after DMA issue instead of waiting for the full 4.7MB.
#  - Partial (tail) m-tiles of adjacent experts are packed into shared
#    PE slots via tile_position col-groups, cutting wasted full-width
#    matmul slots.
#  - Output is stored bf16 (PSUM->SBUF copy casts) and upcast on host.

import os
import sys
import types

import numpy as np


def _wire_ntff_hook():
    """bass_utils' trace path (trace=True or BASS_TRACE=1 under axon)
    imports antenv.axon_hooks, which this image lacks -- the boot degrades
    silently. Recreate the module and register the ctypes NTFF hook so
    tracing works instead of raising ModuleNotFoundError. Best-effort."""
    try:
        import antenv

        if "antenv.axon_hooks" in sys.modules:
            return
        mod = types.ModuleType("antenv.axon_hooks")
        _h = [None]
        mod.set_axon_ntff_profile_hook = lambda h: _h.__setitem__(0, h)
        mod.get_axon_ntff_profile_hook = lambda: _h[0]
        sys.modules["antenv.axon_hooks"] = mod
        antenv.axon_hooks = mod
        try:
            from trn_agent_boot.trn_boot import _ntff_profile_via_ctypes

            mod.set_axon_ntff_profile_hook(
                _ntff_profile_via_ctypes("/opt/axon/libaxon_pjrt.so")
            )
        except Exception:
            pass
    except Exception:
        pass


_wire_ntff_hook()

NUM_TOKENS = 4096
IN_FEATURES = 2560
OUT_FEATURES = 3328
GROUPS = 8
N_CORES = 8
COLS = OUT_FEATURES // N_CORES  # 416
P = 128
K_TILES = IN_FEATURES // P  # 20
CHUNK = 384  # token chunk per x DMA (multiple of 128)
# first-expert W/x DMA k-part sizes: parts gate the first matmuls on
# ~0.9MB slices instead of the full 4MB tile pair
KPARTS = (4, 4, 4, 4, 4)
assert sum(KPARTS) == K_TILES

LAST_EXEC_TIME_NS = None
LAST_TRACE = None

_COMPILED = {}


def _csize(mt):
    # array col-group footprint for a tail of mt tokens. Concurrent
    # col-group matmuls only overlap when their tile_size classes match
    # (mixed 64/32 pairs serialize, measured), so everything <=64 uses
    # the 64 class and packs pair up at positions {0, 64}.
    if mt <= 64:
        return 64
    return 128


def _build(sizes, dt_name, reps=1):
    import concourse.bass as bass
    import concourse.mybir as mybir
    import concourse.tile as tile

    dt_in = getattr(mybir.dt, dt_name)
    f32 = mybir.dt.float32

    nc = bass.Bass()
    xt_d = nc.dram_tensor("xt", [IN_FEATURES, NUM_TOKENS], dt_in, kind="ExternalInput")
    wt_d = nc.dram_tensor(
        "wt", [GROUPS, IN_FEATURES, COLS], dt_in, kind="ExternalInput"
    )
    out_d = nc.dram_tensor("out", [NUM_TOKENS, COLS], dt_in, kind="ExternalOutput")

    offs = [0]
    for s in sizes:
        offs.append(offs[-1] + int(s))

    # k-tiled views: row index (k*128 + p) -> dims [p, k, ...]
    xt_v = xt_d[:, :].rearrange("(k p) t -> p k t", p=P)
    wt_v = wt_d[:, :, :].rearrange("g (k p) c -> g p k c", p=P)

    with tile.TileContext(nc) as tc:
        with (
            tc.tile_pool(name="wp", bufs=5) as wp,
            tc.tile_pool(name="xp", bufs=5) as xp,
            tc.tile_pool(name="pp", bufs=6, space="PSUM") as pp,
            tc.tile_pool(name="op", bufs=3) as op,
        ):
            def body():
                _emit_body(
                    nc, wp, xp, pp, op, sizes, offs, dt_in, f32, xt_v, wt_v, out_d
                )

            if reps > 1:
                with tc.For_i(0, reps, 1):
                    body()
            else:
                body()

    _split_waits(nc, mybir)
    nc.finalize()
    return nc


def _emit_body(nc, wp, xp, pp, op, sizes, offs, dt_in, f32, xt_v, wt_v, out_d):
    # Process experts in descending token count: big experts generate DMA
    # slack that covers the W+x loads of the small experts at the end;
    # ascending/original order starves the PE at early expert boundaries.
    order = sorted(
        (g for g in range(GROUPS) if int(sizes[g]) > 0),
        key=lambda g: -int(sizes[g]),
    )
    if not order:
        return
    first_g = order[0]

    # HAM warm-up: ~4.3us of dependency-free dummy matmuls on garbage SBUF
    # run while the first DMAs are in flight, so the PE clock is at 2.4GHz
    # (not the cold 1.2) when real work starts. PE is idle then anyway.
    warm_w = wp.tile([P, 512], dt_in, tag="warm", bufs=1, name="warm_w")
    warm_ps = pp.tile([64, 512], f32, tag="warmps", bufs=1, name="warm_ps")
    nc.vector.memset(warm_w[:, :], 0.0)
    for _ in range(10):
        nc.tensor.matmul(
            warm_ps[:, :], warm_w[:, :64], warm_w[:, :], start=True, stop=True
        )

    def fillers(n):
        for _ in range(n):
            nc.tensor.matmul(
                warm_ps[:, :], warm_w[:, :64], warm_w[:, :], start=True, stop=True
            )

    # W accessors per expert: wat[g](k) -> AP [P, COLS]
    wat = {}
    # x chunk accessors: xat[(g, c)](k) -> AP [P, clen]
    xat = {}

    def emit_slot(ps_groups, copyspec):
        """ps_groups: list of (pos, mt, xfn, loc, wfn); one psum slot,
        k-accumulated; copyspec: list of (pos, mt, r0) output rows."""
        ps = pp.tile([P, COLS], f32, tag="ps", name=f"ps_{emit_slot.n}")
        packed = len(ps_groups) > 1 or ps_groups[0][0] != 0
        for k in range(K_TILES):
            for pos, mt, xfn, loc, wfn in ps_groups:
                # packed groups are padded to the full 64-col class so all
                # members share one tile_size -- mixed classes serialize.
                # The pad rows compute garbage that is never stored.
                mm = 64 if packed else mt
                nc.tensor.matmul(
                    ps[pos : pos + mm, :],
                    xfn(k)[:, loc : loc + mm],
                    wfn(k),
                    start=(k == 0),
                    stop=(k == K_TILES - 1),
                    tile_position=(0, pos) if packed else None,
                )
        ob = op.tile([P, COLS], dt_in, tag="o", name=f"ob_{emit_slot.n}")
        lo = min(pos for pos, _, _ in copyspec)
        hi = max(pos + mt for pos, mt, _ in copyspec)
        nc.vector.tensor_copy(ob[lo:hi, :], ps[lo:hi, :])
        for pos, mt, r0 in copyspec:
            nc.scalar.dma_start(out_d[r0 : r0 + mt, :], ob[pos : pos + mt, :])
        emit_slot.n += 1

    emit_slot.n = 0

    pend = []  # [(opos, g, pos, mt, xfn, loc, r0)]

    def flush_pend():
        if not pend:
            return
        groups = [(pos, mt, xfn, loc, wat[g]) for _, g, pos, mt, xfn, loc, _ in pend]
        cspec = [(pos, mt, r0) for _, _, pos, mt, _, _, r0 in pend]
        emit_slot(groups, cspec)
        pend.clear()

    def handle_tail(opos, g, off, n_full, mt_tail):
        tail_start = n_full * P
        c_t = tail_start // CHUNK
        loc_t = tail_start - c_t * CHUNK
        xfn_t = xat[(g, c_t)]
        r0_t = off + tail_start
        size = _csize(mt_tail)
        if size == 128:
            emit_slot([(0, mt_tail, xfn_t, loc_t, wat[g])], [(0, mt_tail, r0_t)])
            return
        # try to add to pending pack: find an aligned free col-group
        used = [False] * 4  # 32-col slots
        for _, _, pos, mt, _, _, _ in pend:
            s = _csize(mt)
            for b in range(pos // 32, (pos + s) // 32):
                used[b] = True
        placed = None
        step = size // 32
        for pos in range(0, 128, size):
            blocks = range(pos // 32, pos // 32 + step)
            if all(not used[b] for b in blocks):
                placed = pos
                break
        if placed is None:
            flush_pend()
            placed = 0
        pend.append((opos, g, placed, mt_tail, xfn_t, loc_t, r0_t))

    for opos, g in enumerate(order):
        seg = int(sizes[g])
        off = offs[g]

        # ---- W DMA ----
        if g == first_g:
            w0p = []
            x0p = []
            clen0 = min(CHUNK, seg)
            kmap = []  # k -> (part j, offset within part)
            kbase = 0
            for j, kp in enumerate(KPARTS):
                wt_part = wp.tile(
                    [P, kp, COLS], dt_in, tag="w0", bufs=len(KPARTS), name=f"w0_{j}"
                )
                nc.sync.dma_start(
                    wt_part[:, :, :], wt_v[g][:, kbase : kbase + kp, :]
                )
                w0p.append(wt_part)
                xt_part = xp.tile(
                    [P, kp, CHUNK], dt_in, tag="x0", bufs=len(KPARTS), name=f"x0_{j}"
                )
                nc.sync.dma_start(
                    xt_part[:, :, :clen0],
                    xt_v[:, kbase : kbase + kp, off : off + clen0],
                )
                x0p.append(xt_part)
                for kk in range(kp):
                    kmap.append((j, kk))
                kbase += kp
            wat[g] = lambda k, _w=w0p, _m=kmap: _w[_m[k][0]][:, _m[k][1], :]
            xat[(g, 0)] = lambda k, _x=x0p, _m=kmap: _x[_m[k][0]][:, _m[k][1], :]
        else:
            wtile = wp.tile([P, K_TILES, COLS], dt_in, tag="w", name=f"w_{g}")
            nc.sync.dma_start(wtile[:, :, :], wt_v[g])
            wat[g] = lambda k, _w=wtile: _w[:, k, :]

        # ---- chunks: x DMAs first (Sync-queue order is unchanged, but it
        # lets the last expert's tail pack flush before its full tiles) ----
        n_chunks = (seg + CHUNK - 1) // CHUNK
        n_full = seg // P  # full 128-token m-tiles in segment
        mt_tail = seg % P
        last = opos == len(order) - 1

        for c in range(n_chunks):
            cbase = c * CHUNK
            clen = min(CHUNK, seg - cbase)
            if (g, c) not in xat:
                xtile = xp.tile([P, K_TILES, CHUNK], dt_in, tag="x", name=f"x_{g}_{c}")
                nc.sync.dma_start(
                    xtile[:, :, :clen],
                    xt_v[:, :, off + cbase : off + cbase + clen],
                )
                xat[(g, c)] = lambda k, _x=xtile: _x[:, k, :]

        if last and mt_tail:
            # last expert: flush the tail pack before the full tiles so its
            # copies/stores overlap the remaining compute instead of
            # extending the kernel tail
            handle_tail(opos, g, off, n_full, mt_tail)
            flush_pend()

        for c in range(n_chunks):
            cbase = c * CHUNK
            clen = min(CHUNK, seg - cbase)
            xfn = xat[(g, c)]

            # full m-tiles within this chunk
            m_lo = cbase // P
            m_hi = min(n_full, (cbase + clen) // P)
            if g == first_g and c == 0:
                # part-outer ordering: consume DMA k-parts as they land
                pss = {}
                for m in range(m_lo, m_hi):
                    pss[m] = pp.tile([P, COLS], f32, tag="ps", name=f"ps_f_{m}")
                kbase = 0
                for kp in KPARTS:
                    for m in range(m_lo, m_hi):
                        loc = m * P - cbase
                        for kk in range(kp):
                            k = kbase + kk
                            nc.tensor.matmul(
                                pss[m][:, :],
                                xfn(k)[:, loc : loc + P],
                                wat[g](k),
                                start=(k == 0),
                                stop=(k == K_TILES - 1),
                            )
                    kbase += kp
                for m in range(m_lo, m_hi):
                    ob = op.tile([P, COLS], dt_in, tag="o", name=f"ob_f_{m}")
                    nc.vector.tensor_copy(ob[:, :], pss[m][:, :])
                    r0 = off + m * P
                    nc.scalar.dma_start(out_d[r0 : r0 + P, :], ob[:, :])
                # gap fillers: the next chunk's x DMA lands ~2-4us after the
                # first chunk's compute drains (front is DMA-bound). These
                # dependency-free matmuls run in that bubble, keeping the PE
                # activity window hot so HAM does not re-throttle to 1.2GHz.
                fillers(8)
            else:
                for m in range(m_lo, m_hi):
                    loc = m * P - cbase
                    r0 = off + m * P
                    emit_slot(
                        [(0, P, xfn, loc, wat[g])],
                        [(0, P, r0)],
                    )


        # ---- tail handling (last expert handled before its fulls) ----
        if mt_tail and not last:
            handle_tail(opos, g, off, n_full, mt_tail)

        # flush packs whose oldest member is 2 experts old (keeps the x
        # chunk tiles they reference inside the pool reuse window)
        if pend and min(p[0] for p in pend) <= opos - 2:
            flush_pend()

    flush_pend()


def _split_waits(nc, mybir):
    """This container's walrus build allows at most ONE sync wait per
    instruction ('Too many sync wait commands' otherwise). Split any
    instruction carrying N>1 waits into N-1 same-engine NoOps (one wait
    each) followed by the original instruction with the last wait. Engine
    sequencers execute in order, so semantics are preserved."""
    counter = [0]
    for blk in nc.m.functions[0].blocks:
        insts = blk.instructions
        out = []
        changed = False
        for inst in insts:
            si = inst.sync_info
            if si is not None and len(si.on_wait) > 1:
                waits = list(si.on_wait)
                for w in waits[:-1]:
                    counter[0] += 1
                    nop = mybir.InstNoOp(name=f"I-nopw-{counter[0]}")
                    nop.engine = inst.engine
                    nop.sync_info = mybir.SyncInfo(on_wait=[w], on_update=[])
                    out.append(nop)
                inst.sync_info = mybir.SyncInfo(
                    on_wait=[waits[-1]], on_update=list(si.on_update)
                )
                changed = True
            out.append(inst)
        if changed:
            insts[:] = out


def kernel(input, weight, tokens_per_expert):
    global LAST_EXEC_TIME_NS, LAST_TRACE
    from concourse.bass_utils import run_bass_kernel_spmd

    x = np.asarray(input, dtype=np.float32)
    w = np.asarray(weight, dtype=np.float32)
    sizes = tuple(int(s) for s in np.asarray(tokens_per_expert).reshape(-1))
    assert sum(sizes) == NUM_TOKENS and len(sizes) == GROUPS
    assert x.shape == (NUM_TOKENS, IN_FEATURES)
    assert w.shape == (GROUPS, IN_FEATURES, OUT_FEATURES)

    dt_name = os.environ.get("GG_DTYPE", "bfloat16")
    if dt_name == "bfloat16":
        import ml_dtypes

        np_dt = ml_dtypes.bfloat16
    else:
        np_dt = np.float32

    reps = int(os.environ.get("GG_REPS", "1"))
    key = (sizes, dt_name, reps)
    if key not in _COMPILED:
        _COMPILED[key] = _build(sizes, dt_name, reps)
    nc = _COMPILED[key]

    xt = np.ascontiguousarray(x.T).astype(np_dt)
    in_maps = []
    for c in range(N_CORES):
        wc = np.ascontiguousarray(w[:, :, c * COLS : (c + 1) * COLS]).astype(np_dt)
        in_maps.append({"xt": xt, "wt": wc})

    trace = os.environ.get("GG_TRACE", "0") == "1"
    res = run_bass_kernel_spmd(nc, in_maps, list(range(N_CORES)), trace=trace)
    LAST_EXEC_TIME_NS = res.exec_time_ns
    if res.instructions_and_trace is not None:
        LAST_TRACE = res.instructions_and_trace[1]

    out = np.concatenate(
        [np.asarray(res.results[c]["out"]) for c in range(N_CORES)], axis=1
    ).astype(np.float32)
    return out


# revision 40
# speedup vs baseline: 1.0263x; 1.0263x over previous
# Grouped GEMM (MoE) kernel for Trainium2, 8 NeuronCores.
#
# Sharding: tensor-parallel over out_features (column parallel). Each core
# computes ALL 4096 tokens against its own 416-column slice of every
# expert's weight. No collectives; host concatenates per-core outputs
# along the feature axis. Perfectly load balanced regardless of the
# (uneven) per-expert token counts; the program is identical on every
# core (SPMD) -- only the weight *values* differ.
#
# Layout: the PE contracts over the partition dim of both operands, so
# both need in_features on partitions. The host pre-transposes x once
# (x_T [in, tokens]); w is already [in, out]. Per 128-token tile of one
# expert's segment:
#   psum[tok, col] += xT_tile[k, tok].T @ w_tile[k, col]   (k accumulated)
#
# Perf structure (vs the naive version):
#  - bf16 operands: PE runs 1 cycle/row (fp32 is 4) and DMA bytes halve.
#  - First expert's W and first x chunk are DMA'd in 4 k-part slices and
#    the first chunk's matmuls run part-outer, so the PE starts ~3.5us
#    after DMA issue instead of waiting for the full 4.7MB.
#  - Partial (tail) m-tiles of adjacent experts are packed into shared
#    PE slots via tile_position col-groups, cutting wasted full-width
#    matmul slots.
#  - Output is stored bf16 (PSUM->SBUF copy casts) and upcast on host.

import os
import sys
import types

import numpy as np


def _wire_ntff_hook():
    """bass_utils' trace path (trace=True or BASS_TRACE=1 under axon)
    imports antenv.axon_hooks, which this image lacks -- the boot degrades
    silently. Recreate the module and register the ctypes NTFF hook so
    tracing works instead of raising ModuleNotFoundError. Best-effort."""
    try:
        import antenv

        if "antenv.axon_hooks" in sys.modules:
            return
        mod = types.ModuleType("antenv.axon_hooks")
        _h = [None]
        mod.set_axon_ntff_profile_hook = lambda h: _h.__setitem__(0, h)
        mod.get_axon_ntff_profile_hook = lambda: _h[0]
        sys.modules["antenv.axon_hooks"] = mod
        antenv.axon_hooks = mod
        try:
            from trn_agent_boot.trn_boot import _ntff_profile_via_ctypes

            mod.set_axon_ntff_profile_hook(
                _ntff_profile_via_ctypes("/opt/axon/libaxon_pjrt.so")
            )
        except Exception:
            pass
    except Exception:
        pass


_wire_ntff_hook()

NUM_TOKENS = 4096
IN_FEATURES = 2560
OUT_FEATURES = 3328
GROUPS = 8
N_CORES = 8
COLS = OUT_FEATURES // N_CORES  # 416
P = 128
K_TILES = IN_FEATURES // P  # 20
CHUNK = 384  # token chunk per x DMA (multiple of 128)
# first-expert W/x DMA k-part sizes: parts gate the first matmuls on
# ~0.9MB slices instead of the full 4MB tile pair
KPARTS = (4, 4, 4, 4, 4)
assert sum(KPARTS) == K_TILES

LAST_EXEC_TIME_NS = None
LAST_TRACE = None

_COMPILED = {}


def _csize(mt):
    # array col-group footprint for a tail of mt tokens. Concurrent
    # col-group matmuls only overlap when their tile_size classes match
    # (mixed 64/32 pairs serialize, measured), so everything <=64 uses
    # the 64 class and packs pair up at positions {0, 64}.
    if mt <= 64:
        return 64
    return 128


def _build(sizes, dt_name, reps=1):
    import concourse.bass as bass
    import concourse.mybir as mybir
    import concourse.tile as tile

    dt_in = getattr(mybir.dt, dt_name)
    f32 = mybir.dt.float32

    nc = bass.Bass()
    xt_d = nc.dram_tensor("xt", [IN_FEATURES, NUM_TOKENS], dt_in, kind="ExternalInput")
    wt_d = nc.dram_tensor(
        "wt", [GROUPS, IN_FEATURES, COLS], dt_in, kind="ExternalInput"
    )
    out_d = nc.dram_tensor("out", [NUM_TOKENS, COLS], dt_in, kind="ExternalOutput")

    offs = [0]
    for s in sizes:
        offs.append(offs[-1] + int(s))

    # k-tiled views: row index (k*128 + p) -> dims [p, k, ...]
    xt_v = xt_d[:, :].rearrange("(k p) t -> p k t", p=P)
    wt_v = wt_d[:, :, :].rearrange("g (k p) c -> g p k c", p=P)

    with tile.TileContext(nc) as tc:
        with (
            tc.tile_pool(name="wp", bufs=5) as wp,
            tc.tile_pool(name="xp", bufs=5) as xp,
            tc.tile_pool(name="pp", bufs=6, space="PSUM") as pp,
            tc.tile_pool(name="op", bufs=3) as op,
        ):
            def body():
                _emit_body(
                    nc, wp, xp, pp, op, sizes, offs, dt_in, f32, xt_v, wt_v, out_d
                )

            if reps > 1:
                with tc.For_i(0, reps, 1):
                    body()
            else:
                body()

    _split_waits(nc, mybir)
    nc.finalize()
    return nc


def _emit_body(nc, wp, xp, pp, op, sizes, offs, dt_in, f32, xt_v, wt_v, out_d):
    # Process experts in descending token count: big experts generate DMA
    # slack that covers the W+x loads of the small experts at the end;
    # ascending/original order starves the PE at early expert boundaries.
    order = sorted(
        (g for g in range(GROUPS) if int(sizes[g]) > 0),
        key=lambda g: -int(sizes[g]),
    )
    if not order:
        return
    first_g = order[0]

    # HAM warm-up: ~4.3us of dependency-free dummy matmuls on garbage SBUF
    # run while the first DMAs are in flight, so the PE clock is at 2.4GHz
    # (not the cold 1.2) when real work starts. PE is idle then anyway.
    warm_w = wp.tile([P, 512], dt_in, tag="warm", bufs=1, name="warm_w")
    warm_ps = pp.tile([64, 512], f32, tag="warmps", bufs=1, name="warm_ps")
    nc.vector.memset(warm_w[:, :], 0.0)
    for _ in range(10):
        nc.tensor.matmul(
            warm_ps[:, :], warm_w[:, :64], warm_w[:, :], start=True, stop=True
        )

    def fillers(n):
        for _ in range(n):
            nc.tensor.matmul(
                warm_ps[:, :], warm_w[:, :64], warm_w[:, :], start=True, stop=True
            )

    # W accessors per expert: wat[g](k) -> AP [P, COLS]
    wat = {}
    # x chunk accessors: xat[(g, c)](k) -> AP [P, clen]
    xat = {}
    # raw x chunk tiles (plain, non-part-split chunks only), for tail staging
    xraw = {}

    def emit_slot(ps_groups, copyspec):
        """ps_groups: list of (pos, mt, xfn, loc, wfn); one psum slot,
        k-accumulated; copyspec: list of (pos, mt, r0) output rows."""
        ps = pp.tile([P, COLS], f32, tag="ps", name=f"ps_{emit_slot.n}")
        packed = len(ps_groups) > 1 or ps_groups[0][0] != 0
        for k in range(K_TILES):
            for pos, mt, xfn, loc, wfn in ps_groups:
                # packed groups are padded to the full 64-col class so all
                # members share one tile_size -- mixed classes serialize.
                # The pad rows compute garbage that is never stored.
                mm = 64 if packed else mt
                nc.tensor.matmul(
                    ps[pos : pos + mm, :],
                    xfn(k)[:, loc : loc + mm],
                    wfn(k),
                    start=(k == 0),
                    stop=(k == K_TILES - 1),
                    tile_position=(0, pos) if packed else None,
                )
        ob = op.tile([P, COLS], dt_in, tag="o", name=f"ob_{emit_slot.n}")
        lo = min(pos for pos, _, _ in copyspec)
        hi = max(pos + mt for pos, mt, _ in copyspec)
        nc.vector.tensor_copy(ob[lo:hi, :], ps[lo:hi, :])
        for pos, mt, r0 in copyspec:
            nc.scalar.dma_start(out_d[r0 : r0 + mt, :], ob[pos : pos + mt, :])
        emit_slot.n += 1

    emit_slot.n = 0

    pend = []  # [(opos, g, pos, mt, xfn, loc, r0)]

    def flush_pend():
        if not pend:
            return
        groups = [(pos, mt, xfn, loc, wat[g]) for _, g, pos, mt, xfn, loc, _ in pend]
        cspec = [(pos, mt, r0) for _, _, pos, mt, _, _, r0 in pend]
        emit_slot(groups, cspec)
        pend.clear()

    def handle_tail(opos, g, off, n_full, mt_tail):
        tail_start = n_full * P
        c_t = tail_start // CHUNK
        loc_t = tail_start - c_t * CHUNK
        xfn_t = xat[(g, c_t)]
        r0_t = off + tail_start
        size = _csize(mt_tail)
        if size == 128:
            emit_slot([(0, mt_tail, xfn_t, loc_t, wat[g])], [(0, mt_tail, r0_t)])
            return
        if (g, c_t) in xraw:
            # stage the tail columns into a small tile (cheap DVE copy) so
            # the big x chunk frees as soon as its full tiles are done --
            # otherwise late-flushed tail packs hold the chunk ring and
            # head-of-line-block the last experts' x DMAs
            tb = op.tile([P, K_TILES, 64], dt_in, tag="ts", bufs=3, name=f"ts_{g}")
            xt_tile = xraw[(g, c_t)]
            nc.vector.tensor_copy(tb[:, :, :], xt_tile[:, :, loc_t : loc_t + 64])
            xfn_t = lambda k, _t=tb: _t[:, k, :]
            loc_t = 0
        # try to add to pending pack: find an aligned free col-group
        used = [False] * 4  # 32-col slots
        for _, _, pos, mt, _, _, _ in pend:
            s = _csize(mt)
            for b in range(pos // 32, (pos + s) // 32):
                used[b] = True
        placed = None
        step = size // 32
        for pos in range(0, 128, size):
            blocks = range(pos // 32, pos // 32 + step)
            if all(not used[b] for b in blocks):
                placed = pos
                break
        if placed is None:
            flush_pend()
            placed = 0
        pend.append((opos, g, placed, mt_tail, xfn_t, loc_t, r0_t))

    for opos, g in enumerate(order):
        seg = int(sizes[g])
        off = offs[g]

        # ---- W DMA ----
        if g == first_g:
            kmap = []  # k -> (part j, offset within part)
            kbase = 0
            w0p = []
            x0p = []
            clen0 = min(CHUNK, seg)
            for j, kp in enumerate(KPARTS):
                wt_part = wp.tile(
                    [P, kp, COLS], dt_in, tag="w0", bufs=len(KPARTS), name=f"w0_{j}"
                )
                nc.sync.dma_start(
                    wt_part[:, :, :], wt_v[g][:, kbase : kbase + kp, :]
                )
                w0p.append(wt_part)
                xt_part = xp.tile(
                    [P, kp, CHUNK], dt_in, tag="x0", bufs=len(KPARTS), name=f"x0_{j}"
                )
                nc.sync.dma_start(
                    xt_part[:, :, :clen0],
                    xt_v[:, kbase : kbase + kp, off : off + clen0],
                )
                x0p.append(xt_part)
                for kk in range(kp):
                    kmap.append((j, kk))
                kbase += kp
            wat[g] = lambda k, _w=w0p, _m=kmap: _w[_m[k][0]][:, _m[k][1], :]
            xat[(g, 0)] = lambda k, _x=x0p, _m=kmap: _x[_m[k][0]][:, _m[k][1], :]
            # chunk 1 also lands as k-parts (appended after the c0 parts in
            # the DMA queue) so its matmuls start before the whole chunk
            # arrives -- the c0->c1 transition was a 2.5-3.5us PE stall
            if seg > CHUNK:
                clen1 = min(CHUNK, seg - CHUNK)
                x1p = []
                kbase = 0
                for j, kp in enumerate(KPARTS):
                    xt_part = xp.tile(
                        [P, kp, CHUNK], dt_in, tag="x1", bufs=len(KPARTS),
                        name=f"x1_{j}",
                    )
                    nc.sync.dma_start(
                        xt_part[:, :, :clen1],
                        xt_v[:, kbase : kbase + kp, off + CHUNK : off + CHUNK + clen1],
                    )
                    x1p.append(xt_part)
                    kbase += kp
                xat[(g, 1)] = lambda k, _x=x1p, _m=kmap: _x[_m[k][0]][:, _m[k][1], :]
        else:
            wtile = wp.tile([P, K_TILES, COLS], dt_in, tag="w", name=f"w_{g}")
            nc.sync.dma_start(wtile[:, :, :], wt_v[g])
            wat[g] = lambda k, _w=wtile: _w[:, k, :]

        # ---- chunks: x DMAs first (Sync-queue order is unchanged, but it
        # lets the last expert's tail pack flush before its full tiles) ----
        n_chunks = (seg + CHUNK - 1) // CHUNK
        n_full = seg // P  # full 128-token m-tiles in segment
        mt_tail = seg % P
        last = opos == len(order) - 1

        for c in range(n_chunks):
            cbase = c * CHUNK
            clen = min(CHUNK, seg - cbase)
            if (g, c) not in xat:
                xtile = xp.tile(
                    [P, K_TILES, CHUNK], dt_in, tag="x", bufs=4, name=f"x_{g}_{c}"
                )
                nc.sync.dma_start(
                    xtile[:, :, :clen],
                    xt_v[:, :, off + cbase : off + cbase + clen],
                )
                xat[(g, c)] = lambda k, _x=xtile: _x[:, k, :]
                xraw[(g, c)] = xtile

        if last and mt_tail:
            # last expert: flush the tail pack before the full tiles so its
            # copies/stores overlap the remaining compute instead of
            # extending the kernel tail
            handle_tail(opos, g, off, n_full, mt_tail)
            flush_pend()

        for c in range(n_chunks):
            cbase = c * CHUNK
            clen = min(CHUNK, seg - cbase)
            xfn = xat[(g, c)]

            # full m-tiles within this chunk
            m_lo = cbase // P
            m_hi = min(n_full, (cbase + clen) // P)
            if g == first_g and c <= 1:
                # part-outer ordering: consume DMA k-parts as they land
                pss = {}
                for m in range(m_lo, m_hi):
                    pss[m] = pp.tile([P, COLS], f32, tag="ps", name=f"ps_f_{m}")
                kbase = 0
                for kp in KPARTS:
                    for m in range(m_lo, m_hi):
                        loc = m * P - cbase
                        for kk in range(kp):
                            k = kbase + kk
                            nc.tensor.matmul(
                                pss[m][:, :],
                                xfn(k)[:, loc : loc + P],
                                wat[g](k),
                                start=(k == 0),
                                stop=(k == K_TILES - 1),
                            )
                    kbase += kp
                for m in range(m_lo, m_hi):
                    ob = op.tile([P, COLS], dt_in, tag="o", name=f"ob_f_{m}")
                    nc.vector.tensor_copy(ob[:, :], pss[m][:, :])
                    r0 = off + m * P
                    nc.scalar.dma_start(out_d[r0 : r0 + P, :], ob[:, :])
                if c == min(1, n_chunks - 1):
                    # gap fillers: the next chunk's x DMA lands a couple us
                    # after this compute drains (front is DMA-bound). These
                    # dependency-free matmuls run in that bubble, keeping the
                    # PE activity window hot so HAM does not re-throttle.
                    fillers(8)
            else:
                for m in range(m_lo, m_hi):
                    loc = m * P - cbase
                    r0 = off + m * P
                    emit_slot(
                        [(0, P, xfn, loc, wat[g])],
                        [(0, P, r0)],
                    )


        # ---- tail handling (last expert handled before its fulls) ----
        if mt_tail and not last:
            handle_tail(opos, g, off, n_full, mt_tail)

        # flush packs whose oldest member is 2 experts old (keeps the x
        # chunk tiles they reference inside the pool reuse window)
        if pend and min(p[0] for p in pend) <= opos - 2:
            flush_pend()

    flush_pend()


def _split_waits(nc, mybir):
    """This container's walrus build allows at most ONE sync wait per
    instruction ('Too many sync wait commands' otherwise). Split any
    instruction carrying N>1 waits into N-1 same-engine NoOps (one wait
    each) followed by the original instruction with the last wait. Engine
    sequencers execute in order, so semantics are preserved."""
    counter = [0]
    for blk in nc.m.functions[0].blocks:
        insts = blk.instructions
        out = []
        changed = False
        for inst in insts:
            si = inst.sync_info
            if si is not None and len(si.on_wait) > 1:
                waits = list(si.on_wait)
                for w in waits[:-1]:
                    counter[0] += 1
                    nop = mybir.InstNoOp(name=f"I-nopw-{counter[0]}")
                    nop.engine = inst.engine
                    nop.sync_info = mybir.SyncInfo(on_wait=[w], on_update=[])
                    out.append(nop)
                inst.sync_info = mybir.SyncInfo(
                    on_wait=[waits[-1]], on_update=list(si.on_update)
                )
                changed = True
            out.append(inst)
        if changed:
            insts[:] = out


def kernel(input, weight, tokens_per_expert):
    global LAST_EXEC_TIME_NS, LAST_TRACE
    from concourse.bass_utils import run_bass_kernel_spmd

    x = np.asarray(input, dtype=np.float32)
    w = np.asarray(weight, dtype=np.float32)
    sizes = tuple(int(s) for s in np.asarray(tokens_per_expert).reshape(-1))
    assert sum(sizes) == NUM_TOKENS and len(sizes) == GROUPS
    assert x.shape == (NUM_TOKENS, IN_FEATURES)
    assert w.shape == (GROUPS, IN_FEATURES, OUT_FEATURES)

    dt_name = os.environ.get("GG_DTYPE", "bfloat16")
    if dt_name == "bfloat16":
        import ml_dtypes

        np_dt = ml_dtypes.bfloat16
    else:
        np_dt = np.float32

    reps = int(os.environ.get("GG_REPS", "1"))
    key = (sizes, dt_name, reps)
    if key not in _COMPILED:
        _COMPILED[key] = _build(sizes, dt_name, reps)
    nc = _COMPILED[key]

    xt = np.ascontiguousarray(x.T).astype(np_dt)
    in_maps = []
    for c in range(N_CORES):
        wc = np.ascontiguousarray(w[:, :, c * COLS : (c + 1) * COLS]).astype(np_dt)
        in_maps.append({"xt": xt, "wt": wc})

    trace = os.environ.get("GG_TRACE", "0") == "1"
    res = run_bass_kernel_spmd(nc, in_maps, list(range(N_CORES)), trace=trace)
    LAST_EXEC_TIME_NS = res.exec_time_ns
    if res.instructions_and_trace is not None:
        LAST_TRACE = res.instructions_and_trace[1]

    out = np.concatenate(
        [np.asarray(res.results[c]["out"]) for c in range(N_CORES)], axis=1
    ).astype(np.float32)
    return out
